# revision 50
# baseline (speedup 1.0000x reference)
"""Trainium2 Bass kernel for 5-relation GAT (nn_GAT_76716705841462), v7.

Strategy: destination-sharded, collective-free, fp16 gather table.
  * Host prep (sharding/indexing only): transpose+cast x to fp16, sort each
    relation's edges by destination (self-loops EXCLUDED), bucket into 128-dst
    windows, pad each (window, relation) bucket to (B1+B2)*128 edge slots.
    dma_gather indices are int16 (<32768): B1 "lo" blocks gather from
    T[0:32768], B2 "hi" blocks from T[h0:], flex srcs balance the two.
  * Device phase A (replicated): node table T[n] (768B f16 rows) =
    [h0(128) | 1 | h1(128) | 1 | a_src(2) | a_dst(2) | pad], built in 4-tile
    chunks (batched DMA, alternating sync/scalar HWDGE queues, PSUM copies
    split across ACT/DVE); only the used 262 columns are written.
  * Device phase B, per window (128 dsts), per relation:
      - dma_gathers round-robin over 4 SWDGE queues (num_swdge_queues=4) so
        Q7 descriptor generation runs on 4 core pairs concurrently.
      - ohT[d,e] via PE row-broadcast + f16 is_equal against a static
        partition-index tile; ohf[e,(b,d)] in ONE batched DVE is_equal with
        broadcast APs; a_dst per edge via per-block ohT matmuls (N=2);
        asum = psum + gathered a_src cols (mixed-dtype DVE add).
      - expl = exp(lrelu(asum)) -> f16; two expl-scaled one-hots ohs_h
        (broadcast-AP DVE mults) feed per-head aggregation matmuls whose rhs
        are the RAW gathered rows (numerator + denominator via the 1-cols).
      - Self-loops never gathered: a diag(exp_self) block is appended to the
        aggregation matmul (rhs = the window's own Tw_local rows).
      - out = ps_rel * recip(denom+eps) + R*bias, accumulated across
        relations and stored once per window.
"""

import numpy as np

import concourse.bacc as bacc
import concourse.bass as bass
import concourse.mybir as mybir
import concourse.tile as tile
from concourse.library_config import mlp

P = 128
H = 2
C = 128
D = 256
R = 5
TW = 384          # T row width (f16): 768B, multiple of 256B for dma_gather
USED = 262        # used columns: [h0|1|h1|1|as(2)|ad(2)]
A_OFF = 258       # a_src at 258:260, a_dst at 260:262
EPS = 1e-16
LOW_CAP = 32768

f32 = mybir.dt.float32
f16 = mybir.dt.float16
i16 = mybir.dt.int16

_CACHE = {}
_RUN_KWARGS = {}      # test harness may set e.g. {"trace": True}
_LAST_RESULT = None   # BassKernelResults of the last run (for profiling)


def build_program(n_tiles, w_pc, B1, B2, h0, num_devices):
    import os
    ablate = set(os.environ.get("K_ABLATE", "").split(","))
    BT = B1 + B2
    t_rows = n_tiles * P
    nc = bacc.Bacc("TRN2", target_bir_lowering=False, debug=False,
                   num_devices=num_devices, num_swdge_queues=4)

    xT = nc.dram_tensor("xT", [D, t_rows], f16, kind="ExternalInput")
    xT_local = nc.dram_tensor("xT_local", [D, w_pc * P], f16,
                              kind="ExternalInput")
    Wsrc = nc.dram_tensor("Wsrc", [D, D], f32, kind="ExternalInput")
    Wdst = nc.dram_tensor("Wdst", [D, D], f32, kind="ExternalInput")
    atts = nc.dram_tensor("atts", [1, D], f32, kind="ExternalInput")
    attd = nc.dram_tensor("attd", [1, D], f32, kind="ExternalInput")
    bias_in = nc.dram_tensor("bias_in", [1, D], f32, kind="ExternalInput")
    iota16_in = nc.dram_tensor("iota16_in", [P, P], f16, kind="ExternalInput")
    iotac_in = nc.dram_tensor("iotac_in", [P, 1], f32, kind="ExternalInput")
    lo_cols = R * B1 * P // 16
    hi_cols = R * B2 * P // 16
    lo16 = nc.dram_tensor("lo16", [w_pc * P, lo_cols], i16,
                          kind="ExternalInput")
    hi16 = nc.dram_tensor("hi16", [w_pc * P, hi_cols], i16,
                          kind="ExternalInput")
    drl = nc.dram_tensor("drl", [w_pc * P, R * BT], f16,
                         kind="ExternalInput")
    drlT = nc.dram_tensor("drlT", [w_pc, R * BT * P], f16,
                          kind="ExternalInput")
    y = nc.dram_tensor("y", [w_pc * P, D], f32, kind="ExternalOutput")

    T = nc.dram_tensor("T", [t_rows, TW], f16)
    Tw_local = nc.dram_tensor("Tw_local", [w_pc * P, TW], f16)

    # ---- TileContext 1: setup + table build ----
    with tile.TileContext(nc) as tc:
        with (
            tc.tile_pool(name="setup", bufs=1) as su,
            tc.tile_pool(name="ps_su", bufs=1, space="PSUM") as psu,
        ):
            Exp1 = mybir.ActivationFunctionType.Exp
            ws_h = [su.tile([P, D], f32, name=f"ws_h{k}") for k in range(2)]
            wd_h = [su.tile([P, D], f32, name=f"wd_h{k}") for k in range(2)]
            for k in range(2):
                nc.sync.dma_start(ws_h[k][:], Wsrc[k * P:(k + 1) * P, :])
                nc.sync.dma_start(wd_h[k][:], Wdst[k * P:(k + 1) * P, :])
            ones1 = su.tile([1, P], f32)
            nc.vector.memset(ones1[:], 1.0)
            atts_sb = su.tile([1, D], f32)
            attd_sb = su.tile([1, D], f32)
            nc.sync.dma_start(atts_sb[:], atts[:])
            nc.sync.dma_start(attd_sb[:], attd[:])
            atts_bc = su.tile([P, D], f32)
            attd_bc = su.tile([P, D], f32)
            for row_sb, bc in ((atts_sb, atts_bc), (attd_sb, attd_bc)):
                ps_bc = psu.tile([P, D], f32, name="ps_bc", tag="ps_bc")
                nc.tensor.matmul(out=ps_bc[:], lhsT=ones1[:], rhs=row_sb[:],
                                 start=True, stop=True)
                nc.vector.tensor_copy(bc[:], ps_bc[:])

            # rhs_k[k]: [128(k-part), 262] fp16
            rhs_k = [su.tile([P, USED], f16, name=f"rhs_k{k}")
                     for k in range(2)]
            for k in range(2):
                rk = rhs_k[k]
                nc.vector.memset(rk[:], 0.0)
                nc.vector.tensor_copy(rk[:, 0:C], ws_h[k][:, 0:C])
                nc.vector.tensor_copy(rk[:, C + 1:2 * C + 1], ws_h[k][:, C:D])
                for h in range(H):
                    for src_w, src_bc, col in (
                        (ws_h[k], atts_bc, A_OFF + h),
                        (wd_h[k], attd_bc, A_OFF + 2 + h),
                    ):
                        scratch = su.tile([P, C], f32, name="vscr",
                                          tag="vscr", bufs=2)
                        nc.vector.tensor_tensor(
                            out=scratch[:],
                            in0=src_w[:, h * C:(h + 1) * C],
                            in1=src_bc[:, h * C:(h + 1) * C],
                            op=mybir.AluOpType.mult)
                        rcol = su.tile([P, 1], f32, name="rcol", tag="rcol",
                                       bufs=2)
                        nc.vector.tensor_reduce(
                            out=rcol[:], in_=scratch[:],
                            axis=mybir.AxisListType.X,
                            op=mybir.AluOpType.add)
                        nc.vector.tensor_copy(rk[:, col:col + 1], rcol[:])

            with (
                tc.tile_pool(name="sb_tbl", bufs=3) as stp,
                tc.tile_pool(name="ps_tbl", bufs=4, space="PSUM") as ptp,
            ):
                CHT = 4  # tiles per DMA chunk

                def build_rows(src_dram, dst_dram, t0, nt, eng,
                               with_dst=False):
                    xk = stp.tile([P, D * CHT], f16, name="xk")
                    eng.dma_start(
                        xk[:].rearrange("p (k j c) -> p k j c", k=2, j=CHT)
                        [:, :, 0:nt, :],
                        src_dram[:, t0 * P:(t0 + nt) * P]
                        .rearrange("(k p) (j c) -> p k j c", p=P, j=nt))
                    stg = stp.tile([P, TW * CHT], f16, name="stg")
                    stg3 = stg[:].rearrange("p (j e) -> p j e", e=TW)
                    for j in range(nt):
                        ps_t = ptp.tile([P, USED], f32, name="ps_t")
                        nc.tensor.matmul(out=ps_t[:],
                                         lhsT=xk[:, j * P:(j + 1) * P],
                                         rhs=rhs_k[0][:],
                                         start=True, stop=False)
                        nc.tensor.matmul(
                            out=ps_t[:],
                            lhsT=xk[:, (CHT + j) * P:(CHT + j + 1) * P],
                            rhs=rhs_k[1][:], start=False, stop=True)
                        nc.vector.tensor_copy(stg3[:, j, 0:USED], ps_t[:])
                    nc.vector.memset(stg3[:, 0:nt, C:C + 1], 1.0)
                    nc.vector.memset(stg3[:, 0:nt, 2 * C + 1:2 * C + 2], 1.0)
                    # Es = exp(a_src), Fs = exp(0.2 a_src) at 262:266
                    nc.scalar.activation(stg3[:, 0:nt, 262:264],
                                         stg3[:, 0:nt, 258:260], Exp1)
                    nc.scalar.activation(stg3[:, 0:nt, 264:266],
                                         stg3[:, 0:nt, 258:260], Exp1,
                                         scale=0.2)
                    wout = USED
                    if with_dst:
                        # Ed = exp(a_dst), Fd = exp(0.2 a_dst) at 266:270
                        nc.scalar.activation(stg3[:, 0:nt, 266:268],
                                             stg3[:, 0:nt, 260:262], Exp1)
                        nc.scalar.activation(stg3[:, 0:nt, 268:270],
                                             stg3[:, 0:nt, 260:262], Exp1,
                                             scale=0.2)
                        wout = 270
                    else:
                        wout = 266
                    eng.dma_start(
                        dst_dram[t0 * P:(t0 + nt) * P, 0:wout]
                        .rearrange("(j p) e -> p j e", p=P),
                        stg3[:, 0:nt, 0:wout])

                engs = [nc.sync, nc.scalar]
                for i, t0 in enumerate(range(0, n_tiles, CHT)):
                    build_rows(xT, T, t0, min(CHT, n_tiles - t0),
                               engs[i % 2])
                for i, t0 in enumerate(range(0, w_pc, CHT)):
                    build_rows(xT_local, Tw_local, t0, min(CHT, w_pc - t0),
                               engs[i % 2], with_dst=True)

    # ---- TileContext 2: attention + aggregation ----
    if "nomain" in ablate:
        w_pc = 0
    w_pc = min(w_pc, int(os.environ.get("K_WINCAP", 10**9)))
    with tile.TileContext(nc) as tc:
        with tc.tile_pool(name="su2", bufs=1) as su:
            nc.gpsimd.load_library(mlp)
            iota16 = su.tile([P, P], f16)
            nc.sync.dma_start(iota16[:], iota16_in[:])
            iotac = su.tile([P, 1], f32)
            nc.sync.dma_start(iotac[:], iotac_in[:])
            ones1h = su.tile([1, P], f16)
            nc.vector.memset(ones1h[:], 1.0)
            ones1f = su.tile([1, P], f32)
            nc.vector.memset(ones1f[:], 1.0)
            bias_sb = su.tile([1, D], f32)
            nc.sync.dma_start(bias_sb[:], bias_in[:])
            bias5 = su.tile([P, D], f32)
            with tc.tile_pool(name="ps_su2", bufs=1, space="PSUM") as psu:
                ps_bc = psu.tile([P, D], f32)
                nc.tensor.matmul(out=ps_bc[:], lhsT=ones1f[:], rhs=bias_sb[:],
                                 start=True, stop=True)
                nc.vector.tensor_scalar_mul(bias5[:], ps_bc[:], float(R))

            Exp = mybir.ActivationFunctionType.Exp
            Copy = mybir.ActivationFunctionType.Copy
            Lrelu = mybir.ActivationFunctionType.Lrelu
            ident16 = su.tile([P, P], f16)
            nc.vector.tensor_scalar(
                out=ident16[:], in0=iota16[:], scalar1=iotac[:],
                scalar2=None, op0=mybir.AluOpType.is_equal)
            # iotaPART[d, j] = d (partition index, constant along free)
            zs = su.tile([P, BT * P], f16)
            nc.vector.memset(zs[:], 0.0)
            iotaPART = su.tile([P, BT * P], f16)
            nc.vector.tensor_scalar(
                out=iotaPART[:], in0=zs[:], scalar1=iotac[:],
                scalar2=None, op0=mybir.AluOpType.add)
            gq = [0]  # round-robin SWDGE queue counter
            iotaB = (iota16[:].rearrange("p (a d) -> p a d", a=1)
                     .to_broadcast([P, BT, P]))
            ACT_SCALE = os.environ.get("K_ACTSCALE", "1") == "1"
            STAGE = int(os.environ.get("K_STAGE", "99"))

            inner_pools = (
                tc.tile_pool(name="sb_g", bufs=2),
                tc.tile_pool(name="sb_oh", bufs=6),
                tc.tile_pool(name="sb_gp", bufs=6),
                tc.tile_pool(name="sb_sm", bufs=4),
                tc.tile_pool(name="sb_out", bufs=2),
                tc.tile_pool(name="ps_mm", bufs=2, space="PSUM"),
                tc.tile_pool(name="ps_drl", bufs=2, space="PSUM"),
                tc.tile_pool(name="ps_a", bufs=2, space="PSUM"),
            )
            import contextlib
            stack = contextlib.ExitStack()
            sgp, ohp, gpp, ssp, sop, pmp, pdp, pap = (
                stack.enter_context(p) for p in inner_pools)
            for w in range(w_pc):
                rows = slice(w * P, (w + 1) * P)
                drl_t = ssp.tile([P, R * BT], f16, name="drl_t")
                nc.sync.dma_start(drl_t[:], drl[rows, :])
                drlT_t = ssp.tile([1, R * BT * P], f16, name="drlT_t")
                nc.sync.dma_start(drlT_t[:], drlT[w:w + 1, :])
                lo_t = ssp.tile([P, lo_cols], i16, name="lo_t")
                hi_t = ssp.tile([P, hi_cols], i16, name="hi_t")
                nc.scalar.dma_start(lo_t[:], lo16[rows, :])
                nc.scalar.dma_start(hi_t[:], hi16[rows, :])
                twin = ssp.tile([P, TW], f16, name="twin")
                nc.scalar.dma_start(twin[:], Tw_local[rows, :])

                G_lo = sgp.tile([P, R * B1 * TW], f16, name="G_lo")
                G_hi = sgp.tile([P, R * B2 * TW], f16, name="G_hi")
                if "nogather" in ablate:
                    nc.vector.memset(G_lo[:], 0.25)
                    nc.vector.memset(G_hi[:], 0.25)
                else:
                    CH = 8 * P  # 1024 idx = 64 descriptors/engine (HW limit)
                    for tot, gt, tbl in ((R * B1 * P, G_lo, T[0:LOW_CAP, :]),
                                         (R * B2 * P, G_hi,
                                          T[h0:t_rows, :])):
                        it = lo_t if gt is G_lo else hi_t
                        off = 0
                        while off < tot:
                            n = min(CH, tot - off)
                            nc.gpsimd.dma_gather(
                                out_ap=gt[:, (off // P) * TW:
                                           ((off + n) // P) * TW]
                                    .rearrange("p (j e) -> p j e", e=TW),
                                in_ap=tbl,
                                idxs_ap=it[:, off // 16:(off + n) // 16],
                                num_idxs=n,
                                num_idxs_reg=n,
                                elem_size=TW,
                                queue_num=gq[0] % 4)
                            gq[0] += 1
                            off += n
                G_lo3 = G_lo[:].rearrange("p (j e) -> p j e", e=TW)
                G_hi3 = G_hi[:].rearrange("p (j e) -> p j e", e=TW)

                # window-local self-loop stats
                m4 = ssp.tile([P, 4], f32, name="m4")
                nc.vector.tensor_tensor(
                    out=m4[:], in0=twin[:, 262:266],
                    in1=twin[:, 266:270], op=mybir.AluOpType.mult)
                expl_self = ssp.tile([P, H], f32, name="expl_self")
                nc.vector.tensor_tensor(
                    out=expl_self[:], in0=m4[:, 0:2], in1=m4[:, 2:4],
                    op=mybir.AluOpType.max)
                diag_es = []
                for h in range(H):
                    dg = ssp.tile([P, P], f16, name=f"diag{h}",
                                  tag=f"diag{h}")
                    nc.vector.tensor_scalar(
                        out=dg[:], in0=ident16[:],
                        scalar1=expl_self[:, h:h + 1], scalar2=None,
                        op0=mybir.AluOpType.mult)
                    diag_es.append(dg)

                outacc = sop.tile([P, D], f32, name="outacc")

                for r in range(R):
                    if "norel" in ablate or STAGE < 2:
                        continue
                    # ohT[d, b*128+e] = (d == dst_local(slot(b,e)))
                    ps_drl = pdp.tile([P, BT * P], f32, name="ps_drl")
                    base = r * BT * P
                    off = 0
                    while off < BT * P:
                        seg = min(512, BT * P - off)
                        nc.tensor.matmul(
                            out=ps_drl[:, off:off + seg],
                            lhsT=ones1h[:],
                            rhs=drlT_t[:, base + off:base + off + seg],
                            start=True, stop=True)
                        off += seg
                    drlF = ohp.tile([P, BT * P], f16, name="drlF", tag="drlF")
                    nc.scalar.copy(drlF[:], ps_drl[:])
                    ohT = ohp.tile([P, BT * P], f16, name="ohT")
                    nc.vector.tensor_tensor(
                        out=ohT[:], in0=drlF[:], in1=iotaPART[:],
                        op=mybir.AluOpType.is_equal)

                    # ohf[e, b*128+d] = (dloc(b,e) == d), all blocks batched
                    ohf = ohp.tile([P, BT * P], f16, name="ohf")
                    drlB = (drl_t[:, r * BT:(r + 1) * BT]
                            .rearrange("p (b a) -> p b a", a=1)
                            .to_broadcast([P, BT, P]))
                    nc.vector.tensor_tensor(
                        out=ohf[:].rearrange("p (b d) -> p b d", d=P),
                        in0=iotaB, in1=drlB, op=mybir.AluOpType.is_equal)

                    # a_dst per edge via ohT matmuls; asum = a_dst + a_src
                    ps_a = pap.tile([P, BT * 4], f32, name="ps_a")
                    for b in range(BT):
                        nc.tensor.matmul(
                            out=ps_a[:, b * 4:(b + 1) * 4],
                            lhsT=ohT[:, b * P:(b + 1) * P],
                            rhs=twin[:, 266:270],
                            start=True, stop=True)
                    prod = ssp.tile([P, BT * 4], f32, name="prod")
                    nc.vector.tensor_tensor(
                        out=prod[:, 0:B1 * 4], in0=ps_a[:, 0:B1 * 4],
                        in1=G_lo3[:, r * B1:(r + 1) * B1, 262:266],
                        op=mybir.AluOpType.mult)
                    nc.vector.tensor_tensor(
                        out=prod[:, B1 * 4:BT * 4], in0=ps_a[:, B1 * 4:BT * 4],
                        in1=G_hi3[:, r * B2:(r + 1) * B2, 262:266],
                        op=mybir.AluOpType.mult)
                    prod4 = prod[:].rearrange("p (b k) -> p b k", k=4)
                    expl = ssp.tile([P, BT * H], f16, name="expl")
                    nc.vector.tensor_tensor(
                        out=expl[:].rearrange("p (b h) -> p b h", h=H),
                        in0=prod4[:, :, 0:2], in1=prod4[:, :, 2:4],
                        op=mybir.AluOpType.max)

                    if STAGE < 3:
                        continue
                    # ohs_h[e, b, d] = ohf * expl[e, b, h]; agg rhs = raw G
                    expl3 = expl[:].rearrange("p (b h) -> p b h", h=H)
                    ps_rel = pmp.tile([P, 2 * (C + 1)], f32, name="ps_rel")
                    for h in range(H):
                        ohs = gpp.tile([P, BT * P], f16, name="ohs",
                                       tag="ohs")
                        nc.vector.tensor_tensor(
                            out=ohs[:].rearrange("p (b d) -> p b d", d=P),
                            in0=ohf[:].rearrange("p (b d) -> p b d", d=P),
                            in1=expl3[:, :, h:h + 1].to_broadcast([P, BT, P]),
                            op=mybir.AluOpType.mult)
                        for b in range(BT):
                            if b < B1:
                                grhs = G_lo3[:, r * B1 + b,
                                             h * (C + 1):(h + 1) * (C + 1)]
                            else:
                                grhs = G_hi3[:, r * B2 + (b - B1),
                                             h * (C + 1):(h + 1) * (C + 1)]
                            nc.tensor.matmul(
                                out=ps_rel[:, h * (C + 1):(h + 1) * (C + 1)],
                                lhsT=ohs[:, b * P:(b + 1) * P],
                                rhs=grhs,
                                start=(b == 0), stop=False)
                        # self-loop: diag(expl_self_h) @ twin half
                        nc.tensor.matmul(
                            out=ps_rel[:, h * (C + 1):(h + 1) * (C + 1)],
                            lhsT=diag_es[h][:],
                            rhs=twin[:, h * (C + 1):(h + 1) * (C + 1)],
                            start=False, stop=True)

                    # divide + self-loop contribution + accumulate
                    if STAGE < 4:
                        continue
                    s_eps = ssp.tile([P, H], f32, name="s_eps")
                    nc.vector.tensor_scalar(
                        out=s_eps[:].rearrange("p (h a) -> p h a", a=1),
                        in0=ps_rel[:].rearrange("p (h c) -> p h c",
                                                c=C + 1)[:, :, C:C + 1],
                        scalar1=EPS, scalar2=None,
                        op0=mybir.AluOpType.add)
                    recip = ssp.tile([P, H], f32, name="recip")
                    nc.vector.reciprocal(recip[:], s_eps[:])
                    for h in range(H):
                        acc_in = (bias5 if r == 0 else outacc)
                        nc.vector.scalar_tensor_tensor(
                            out=outacc[:, h * C:(h + 1) * C],
                            in0=ps_rel[:, h * (C + 1):h * (C + 1) + C],
                            scalar=recip[:, h:h + 1],
                            in1=acc_in[:, h * C:(h + 1) * C],
                            op0=mybir.AluOpType.mult,
                            op1=mybir.AluOpType.add)
                if "norel" in ablate or STAGE < 4:
                    nc.vector.memset(outacc[:], 0.0)
                nc.sync.dma_start(y[rows, :], outacc[:])
            stack.close()

    nc.finalize()
    return nc


def _wrap16(vals):
    """[n] int array -> 16-partition-wrapped [128, n//16] int16 (replicated)."""
    n = len(vals)
    assert n % 16 == 0
    a = np.asarray(vals, np.int16).reshape(n // 16, 16).T  # [16, n//16]
    return np.tile(a, (8, 1))


def prep_inputs(inputs, ncores):
    x = np.asarray(inputs["x"], dtype=np.float32)
    N = x.shape[0]
    nw_real = -(-N // P)
    NW = -(-nw_real // ncores) * ncores
    w_pc = NW // ncores
    n_tiles = NW
    t_rows = NW * P
    low_cap = min(LOW_CAP, t_rows)
    h0 = t_rows - low_cap

    rels = ["parent", "child", "precede", "follow", "peer"]
    per_rel = []
    for rn in rels:
        ei = np.asarray(inputs[f"edge_index_{rn}"])
        src = ei[0].astype(np.int64)
        dst = ei[1].astype(np.int64)
        order = np.argsort(dst, kind="stable")
        src, dst = src[order], dst[order]
        w_of = dst // P
        cnt = np.bincount(w_of, minlength=NW)
        starts = np.zeros(NW + 1, np.int64)
        np.cumsum(cnt, out=starts[1:])
        per_rel.append((src, dst, starts))

    # global B1/B2 from per-(w,r) counts
    must_lo_max = must_hi_max = tot_max = 0
    for src, dst, starts in per_rel:
        for w in range(NW):
            s, e = starts[w], starts[w + 1]
            sw = src[s:e]
            must_lo_max = max(must_lo_max, int((sw < h0).sum()))
            must_hi_max = max(must_hi_max, int((sw >= low_cap).sum()))
            tot_max = max(tot_max, e - s)
    B1 = max(1, -(-must_lo_max // P))
    B2 = max(1, -(-must_hi_max // P), -(-tot_max // P) - B1)
    while B1 * P < must_lo_max or (tot_max - B1 * P) > B2 * P:
        B1 += 1
    BT = B1 + B2

    lo_idx = np.zeros((NW, R, B1 * P), np.int64)
    hi_idx = np.zeros((NW, R, B2 * P), np.int64)  # pad -> hi row 0 (valid)
    dloc = np.full((NW, R, BT * P), float(P), np.float32)  # sentinel 128
    for r, (src, dst, starts) in enumerate(per_rel):
        for w in range(NW):
            s, e = starts[w], starts[w + 1]
            sw, dw = src[s:e], dst[s:e]
            is_lo = sw < h0
            is_hi = sw >= low_cap
            flex = ~is_lo & ~is_hi
            n_lo_strict = int(is_lo.sum())
            room = B1 * P - n_lo_strict
            fi = np.flatnonzero(flex)
            lo_sel = np.concatenate([np.flatnonzero(is_lo), fi[:room]])
            hi_sel = np.concatenate([np.flatnonzero(is_hi), fi[room:]])
            assert len(lo_sel) <= B1 * P and len(hi_sel) <= B2 * P, (
                w, r, len(lo_sel), len(hi_sel))
            lo_idx[w, r, :len(lo_sel)] = sw[lo_sel]
            hi_idx[w, r, :len(hi_sel)] = sw[hi_sel] - h0
            dloc[w, r, :len(lo_sel)] = (dw[lo_sel] - w * P)
            dloc[w, r, B1 * P:B1 * P + len(hi_sel)] = (dw[hi_sel] - w * P)

    xTf = np.zeros((D, t_rows), np.float16)
    xTf[:, :N] = np.ascontiguousarray(x.T).astype(np.float16)

    iota16 = np.tile(np.arange(P, dtype=np.float16), (P, 1))
    iotac = np.arange(P, dtype=np.float32).reshape(P, 1)

    shared = {
        "xT": xTf,
        "Wsrc": np.ascontiguousarray(np.asarray(inputs["W_src"], np.float32)),
        "Wdst": np.ascontiguousarray(np.asarray(inputs["W_dst"], np.float32)),
        "atts": np.asarray(inputs["att_src"], np.float32).reshape(1, D).copy(),
        "attd": np.asarray(inputs["att_dst"], np.float32).reshape(1, D).copy(),
        "bias_in": np.asarray(inputs["bias"], np.float32).reshape(1, D).copy(),
        "iota16_in": iota16,
        "iotac_in": iotac,
    }

    percore = []
    for c in range(ncores):
        lo16 = np.zeros((w_pc * P, R * B1 * P // 16), np.int16)
        hi16 = np.zeros((w_pc * P, R * B2 * P // 16), np.int16)
        drl16 = np.zeros((w_pc * P, R * BT), np.float16)
        drlT16 = np.zeros((w_pc, R * BT * P), np.float16)
        for wl in range(w_pc):
            w = c * w_pc + wl
            lo_vals = np.concatenate([lo_idx[w, r] for r in range(R)])
            hi_vals = np.concatenate([hi_idx[w, r] for r in range(R)])
            lo16[wl * P:(wl + 1) * P, :] = _wrap16(lo_vals)
            hi16[wl * P:(wl + 1) * P, :] = _wrap16(hi_vals)
            # drl16[p, r*BT+b] = dloc[w, r, b*128+p]
            drl16[wl * P:(wl + 1) * P, :] = dloc[w].reshape(R * BT, P).T
            drlT16[wl, :] = dloc[w].reshape(-1)
        percore.append({
            "lo16": lo16, "hi16": hi16, "drl": drl16, "drlT": drlT16,
            "xT_local": np.ascontiguousarray(
                xTf[:, c * w_pc * P:(c + 1) * w_pc * P]),
        })
    meta = dict(N=N, NW=NW, w_pc=w_pc, n_tiles=n_tiles, B1=B1, B2=B2, h0=h0)
    return meta, shared, percore


def kernel(**inputs):
    global _LAST_RESULT
    from concourse.bass_utils import run_bass_kernel_spmd

    ncores = 8
    meta, shared, percore = prep_inputs(inputs, ncores)
    key = tuple(sorted(meta.items()))
    if key not in _CACHE:
        _CACHE[key] = build_program(
            meta["n_tiles"], meta["w_pc"], meta["B1"], meta["B2"],
            meta["h0"], ncores)
    nc = _CACHE[key]
    in_maps = [dict(shared, **percore[c]) for c in range(ncores)]
    res = run_bass_kernel_spmd(nc, in_maps, core_ids=list(range(ncores)),
                               **_RUN_KWARGS)
    _LAST_RESULT = res
    out = np.concatenate([res.results[c]["y"] for c in range(ncores)], axis=0)
    return np.ascontiguousarray(out[:meta["N"]])



# revision 51
# speedup vs baseline: 1.3709x; 1.3709x over previous
"""Trainium2 Bass kernel for 5-relation GAT (nn_GAT_76716705841462), v7.

Strategy: destination-sharded, collective-free, fp16 gather table.
  * Host prep (sharding/indexing only): transpose+cast x to fp16, sort each
    relation's edges by destination (self-loops EXCLUDED), bucket into 128-dst
    windows, pad each (window, relation) bucket to (B1+B2)*128 edge slots.
    dma_gather indices are int16 (<32768): B1 "lo" blocks gather from
    T[0:32768], B2 "hi" blocks from T[h0:], flex srcs balance the two.
  * Device phase A (replicated): node table T[n] (768B f16 rows) =
    [h0(128) | 1 | h1(128) | 1 | a_src(2) | a_dst(2) | pad], built in 4-tile
    chunks (batched DMA, alternating sync/scalar HWDGE queues, PSUM copies
    split across ACT/DVE); only the used 262 columns are written.
  * Device phase B, per window (128 dsts), per relation:
      - dma_gathers round-robin over 4 SWDGE queues (num_swdge_queues=4) so
        Q7 descriptor generation runs on 4 core pairs concurrently.
      - ohT[d,e] via PE row-broadcast + f16 is_equal against a static
        partition-index tile; ohf[e,(b,d)] in ONE batched DVE is_equal with
        broadcast APs; a_dst per edge via per-block ohT matmuls (N=2);
        asum = psum + gathered a_src cols (mixed-dtype DVE add).
      - expl = exp(lrelu(asum)) -> f16; two expl-scaled one-hots ohs_h
        (broadcast-AP DVE mults) feed per-head aggregation matmuls whose rhs
        are the RAW gathered rows (numerator + denominator via the 1-cols).
      - Self-loops never gathered: a diag(exp_self) block is appended to the
        aggregation matmul (rhs = the window's own Tw_local rows).
      - out = ps_rel * recip(denom+eps) + R*bias, accumulated across
        relations and stored once per window.
"""

import numpy as np

import concourse.bacc as bacc
import concourse.bass as bass
import concourse.mybir as mybir
import concourse.tile as tile
from concourse.library_config import mlp

P = 128
H = 2
C = 128
D = 256
R = 5
TW = 384          # T row width (f16): 768B, multiple of 256B for dma_gather
USED = 262        # used columns: [h0|1|h1|1|as(2)|ad(2)]
A_OFF = 258       # a_src at 258:260, a_dst at 260:262
EPS = 1e-16
LOW_CAP = 32768

f32 = mybir.dt.float32
f16 = mybir.dt.float16
i16 = mybir.dt.int16

_CACHE = {}
_RUN_KWARGS = {}      # test harness may set e.g. {"trace": True}
_LAST_RESULT = None   # BassKernelResults of the last run (for profiling)


def build_program(n_tiles, w_pc, B1, B2, h0, num_devices):
    import os
    ablate = set(os.environ.get("K_ABLATE", "").split(","))
    BT = B1 + B2
    t_rows = n_tiles * P
    nc = bacc.Bacc("TRN2", target_bir_lowering=False, debug=False,
                   num_devices=num_devices, num_swdge_queues=4)

    xT = nc.dram_tensor("xT", [D, t_rows], f16, kind="ExternalInput")
    xT_local = nc.dram_tensor("xT_local", [D, w_pc * P], f16,
                              kind="ExternalInput")
    Wsrc = nc.dram_tensor("Wsrc", [D, D], f32, kind="ExternalInput")
    Wdst = nc.dram_tensor("Wdst", [D, D], f32, kind="ExternalInput")
    atts = nc.dram_tensor("atts", [1, D], f32, kind="ExternalInput")
    attd = nc.dram_tensor("attd", [1, D], f32, kind="ExternalInput")
    bias_in = nc.dram_tensor("bias_in", [1, D], f32, kind="ExternalInput")
    iota16_in = nc.dram_tensor("iota16_in", [P, P], f16, kind="ExternalInput")
    iotac_in = nc.dram_tensor("iotac_in", [P, 1], f32, kind="ExternalInput")
    lo_cols = R * B1 * P // 16
    hi_cols = R * B2 * P // 16
    lo16 = nc.dram_tensor("lo16", [w_pc * P, lo_cols], i16,
                          kind="ExternalInput")
    hi16 = nc.dram_tensor("hi16", [w_pc * P, hi_cols], i16,
                          kind="ExternalInput")
    drl = nc.dram_tensor("drl", [w_pc * P, R * BT], f16,
                         kind="ExternalInput")
    drlT = nc.dram_tensor("drlT", [w_pc, R * BT * P], f16,
                          kind="ExternalInput")
    y = nc.dram_tensor("y", [w_pc * P, D], f32, kind="ExternalOutput")

    T = nc.dram_tensor("T", [t_rows, TW], f16)
    Tw_local = nc.dram_tensor("Tw_local", [w_pc * P, TW], f16)

    # ---- TileContext 1: setup + table build ----
    with tile.TileContext(nc) as tc:
        with (
            tc.tile_pool(name="setup", bufs=1) as su,
            tc.tile_pool(name="ps_su", bufs=1, space="PSUM") as psu,
        ):
            Exp1 = mybir.ActivationFunctionType.Exp
            ws_h = [su.tile([P, D], f32, name=f"ws_h{k}") for k in range(2)]
            wd_h = [su.tile([P, D], f32, name=f"wd_h{k}") for k in range(2)]
            for k in range(2):
                nc.sync.dma_start(ws_h[k][:], Wsrc[k * P:(k + 1) * P, :])
                nc.sync.dma_start(wd_h[k][:], Wdst[k * P:(k + 1) * P, :])
            ones1 = su.tile([1, P], f32)
            nc.vector.memset(ones1[:], 1.0)
            atts_sb = su.tile([1, D], f32)
            attd_sb = su.tile([1, D], f32)
            nc.sync.dma_start(atts_sb[:], atts[:])
            nc.sync.dma_start(attd_sb[:], attd[:])
            atts_bc = su.tile([P, D], f32)
            attd_bc = su.tile([P, D], f32)
            for row_sb, bc in ((atts_sb, atts_bc), (attd_sb, attd_bc)):
                ps_bc = psu.tile([P, D], f32, name="ps_bc", tag="ps_bc")
                nc.tensor.matmul(out=ps_bc[:], lhsT=ones1[:], rhs=row_sb[:],
                                 start=True, stop=True)
                nc.vector.tensor_copy(bc[:], ps_bc[:])

            # rhs_k[k]: [128(k-part), 262] fp16
            rhs_k = [su.tile([P, USED], f16, name=f"rhs_k{k}")
                     for k in range(2)]
            for k in range(2):
                rk = rhs_k[k]
                nc.vector.memset(rk[:], 0.0)
                nc.vector.tensor_copy(rk[:, 0:C], ws_h[k][:, 0:C])
                nc.vector.tensor_copy(rk[:, C + 1:2 * C + 1], ws_h[k][:, C:D])
                for h in range(H):
                    for src_w, src_bc, col in (
                        (ws_h[k], atts_bc, A_OFF + h),
                        (wd_h[k], attd_bc, A_OFF + 2 + h),
                    ):
                        scratch = su.tile([P, C], f32, name="vscr",
                                          tag="vscr", bufs=2)
                        nc.vector.tensor_tensor(
                            out=scratch[:],
                            in0=src_w[:, h * C:(h + 1) * C],
                            in1=src_bc[:, h * C:(h + 1) * C],
                            op=mybir.AluOpType.mult)
                        rcol = su.tile([P, 1], f32, name="rcol", tag="rcol",
                                       bufs=2)
                        nc.vector.tensor_reduce(
                            out=rcol[:], in_=scratch[:],
                            axis=mybir.AxisListType.X,
                            op=mybir.AluOpType.add)
                        nc.vector.tensor_copy(rk[:, col:col + 1], rcol[:])

            with (
                tc.tile_pool(name="sb_tbl", bufs=3) as stp,
                tc.tile_pool(name="ps_tbl", bufs=4, space="PSUM") as ptp,
            ):
                CHT = 4  # tiles per DMA chunk

                def build_rows(src_dram, dst_dram, t0, nt, eng,
                               with_dst=False):
                    xk = stp.tile([P, D * CHT], f16, name="xk")
                    eng.dma_start(
                        xk[:].rearrange("p (k j c) -> p k j c", k=2, j=CHT)
                        [:, :, 0:nt, :],
                        src_dram[:, t0 * P:(t0 + nt) * P]
                        .rearrange("(k p) (j c) -> p k j c", p=P, j=nt))
                    stg = stp.tile([P, TW * CHT], f16, name="stg")
                    stg3 = stg[:].rearrange("p (j e) -> p j e", e=TW)
                    for j in range(nt):
                        ps_t = ptp.tile([P, USED], f32, name="ps_t")
                        nc.tensor.matmul(out=ps_t[:],
                                         lhsT=xk[:, j * P:(j + 1) * P],
                                         rhs=rhs_k[0][:],
                                         start=True, stop=False)
                        nc.tensor.matmul(
                            out=ps_t[:],
                            lhsT=xk[:, (CHT + j) * P:(CHT + j + 1) * P],
                            rhs=rhs_k[1][:], start=False, stop=True)
                        nc.vector.tensor_copy(stg3[:, j, 0:USED], ps_t[:])
                    nc.vector.memset(stg3[:, 0:nt, C:C + 1], 1.0)
                    nc.vector.memset(stg3[:, 0:nt, 2 * C + 1:2 * C + 2], 1.0)
                    # Es = exp(a_src), Fs = exp(0.2 a_src) at 262:266
                    nc.scalar.activation(stg3[:, 0:nt, 262:264],
                                         stg3[:, 0:nt, 258:260], Exp1)
                    nc.scalar.activation(stg3[:, 0:nt, 264:266],
                                         stg3[:, 0:nt, 258:260], Exp1,
                                         scale=0.2)
                    wout = USED
                    if with_dst:
                        # Ed = exp(a_dst), Fd = exp(0.2 a_dst) at 266:270
                        nc.scalar.activation(stg3[:, 0:nt, 266:268],
                                             stg3[:, 0:nt, 260:262], Exp1)
                        nc.scalar.activation(stg3[:, 0:nt, 268:270],
                                             stg3[:, 0:nt, 260:262], Exp1,
                                             scale=0.2)
                        wout = 270
                    else:
                        wout = 266
                    eng.dma_start(
                        dst_dram[t0 * P:(t0 + nt) * P, 0:wout]
                        .rearrange("(j p) e -> p j e", p=P),
                        stg3[:, 0:nt, 0:wout])

                engs = [nc.sync, nc.scalar]
                for i, t0 in enumerate(range(0, n_tiles, CHT)):
                    build_rows(xT, T, t0, min(CHT, n_tiles - t0),
                               engs[i % 2])
                for i, t0 in enumerate(range(0, w_pc, CHT)):
                    build_rows(xT_local, Tw_local, t0, min(CHT, w_pc - t0),
                               engs[i % 2], with_dst=True)

    # ---- TileContext 2: attention + aggregation ----
    if "nomain" in ablate:
        w_pc = 0
    w_pc = min(w_pc, int(os.environ.get("K_WINCAP", 10**9)))
    with tile.TileContext(nc) as tc:
        with tc.tile_pool(name="su2", bufs=1) as su:
            nc.gpsimd.load_library(mlp)
            iota16 = su.tile([P, P], f16)
            nc.sync.dma_start(iota16[:], iota16_in[:])
            iotac = su.tile([P, 1], f32)
            nc.sync.dma_start(iotac[:], iotac_in[:])
            ones1h = su.tile([1, P], f16)
            nc.vector.memset(ones1h[:], 1.0)
            ones1f = su.tile([1, P], f32)
            nc.vector.memset(ones1f[:], 1.0)
            bias_sb = su.tile([1, D], f32)
            nc.sync.dma_start(bias_sb[:], bias_in[:])
            bias5 = su.tile([P, D], f32)
            with tc.tile_pool(name="ps_su2", bufs=1, space="PSUM") as psu:
                ps_bc = psu.tile([P, D], f32)
                nc.tensor.matmul(out=ps_bc[:], lhsT=ones1f[:], rhs=bias_sb[:],
                                 start=True, stop=True)
                nc.vector.tensor_scalar_mul(bias5[:], ps_bc[:], float(R))

            Exp = mybir.ActivationFunctionType.Exp
            Copy = mybir.ActivationFunctionType.Copy
            Lrelu = mybir.ActivationFunctionType.Lrelu
            ident16 = su.tile([P, P], f16)
            nc.vector.tensor_scalar(
                out=ident16[:], in0=iota16[:], scalar1=iotac[:],
                scalar2=None, op0=mybir.AluOpType.is_equal)
            # iotaPART[d, j] = d (partition index, constant along free)
            zs = su.tile([P, BT * P], f16)
            nc.vector.memset(zs[:], 0.0)
            iotaPART = su.tile([P, BT * P], f16)
            nc.vector.tensor_scalar(
                out=iotaPART[:], in0=zs[:], scalar1=iotac[:],
                scalar2=None, op0=mybir.AluOpType.add)
            gq = [0]  # round-robin SWDGE queue counter
            iotaB = (iota16[:].rearrange("p (a d) -> p a d", a=1)
                     .to_broadcast([P, BT, P]))
            ACT_SCALE = os.environ.get("K_ACTSCALE", "1") == "1"
            STAGE = int(os.environ.get("K_STAGE", "99"))

            inner_pools = (
                tc.tile_pool(name="sb_g", bufs=2),
                tc.tile_pool(name="sb_oh", bufs=6),
                tc.tile_pool(name="sb_gp", bufs=6),
                tc.tile_pool(name="sb_sm", bufs=3),
                tc.tile_pool(name="sb_out", bufs=2),
                tc.tile_pool(name="ps_mm", bufs=2, space="PSUM"),
                tc.tile_pool(name="ps_drl", bufs=2, space="PSUM"),
                tc.tile_pool(name="ps_a", bufs=2, space="PSUM"),
            )
            import contextlib
            stack = contextlib.ExitStack()
            sgp, ohp, gpp, ssp, sop, pmp, pdp, pap = (
                stack.enter_context(p) for p in inner_pools)
            for w in range(w_pc):
                rows = slice(w * P, (w + 1) * P)
                drl_t = ssp.tile([P, R * BT], f16, name="drl_t")
                nc.sync.dma_start(drl_t[:], drl[rows, :])
                drlT_t = ssp.tile([1, R * BT * P], f16, name="drlT_t")
                nc.sync.dma_start(drlT_t[:], drlT[w:w + 1, :])
                lo_t = ssp.tile([P, lo_cols], i16, name="lo_t")
                hi_t = ssp.tile([P, hi_cols], i16, name="hi_t")
                nc.sync.dma_start(lo_t[:], lo16[rows, :])
                nc.sync.dma_start(hi_t[:], hi16[rows, :])
                twin = ssp.tile([P, TW], f16, name="twin")
                nc.sync.dma_start(twin[:], Tw_local[rows, :])

                G_lo = sgp.tile([P, R * B1 * TW], f16, name="G_lo")
                G_hi = sgp.tile([P, R * B2 * TW], f16, name="G_hi")
                if "nogather" in ablate:
                    nc.vector.memset(G_lo[:], 0.25)
                    nc.vector.memset(G_hi[:], 0.25)
                else:
                    CH = 8 * P  # 1024 idx = 64 descriptors/engine (HW limit)
                    for tot, gt, tbl in ((R * B1 * P, G_lo, T[0:LOW_CAP, :]),
                                         (R * B2 * P, G_hi,
                                          T[h0:t_rows, :])):
                        it = lo_t if gt is G_lo else hi_t
                        off = 0
                        while off < tot:
                            n = min(CH, tot - off)
                            nc.gpsimd.dma_gather(
                                out_ap=gt[:, (off // P) * TW:
                                           ((off + n) // P) * TW]
                                    .rearrange("p (j e) -> p j e", e=TW),
                                in_ap=tbl,
                                idxs_ap=it[:, off // 16:(off + n) // 16],
                                num_idxs=n,
                                num_idxs_reg=n,
                                elem_size=TW,
                                queue_num=gq[0] % 4)
                            gq[0] += 1
                            off += n
                G_lo3 = G_lo[:].rearrange("p (j e) -> p j e", e=TW)
                G_hi3 = G_hi[:].rearrange("p (j e) -> p j e", e=TW)

                # window-local self-loop stats
                m4 = ssp.tile([P, 4], f32, name="m4")
                nc.vector.tensor_tensor(
                    out=m4[:], in0=twin[:, 262:266],
                    in1=twin[:, 266:270], op=mybir.AluOpType.mult)
                expl_self = ssp.tile([P, H], f32, name="expl_self")
                nc.vector.tensor_tensor(
                    out=expl_self[:], in0=m4[:, 0:2], in1=m4[:, 2:4],
                    op=mybir.AluOpType.max)
                diag_es = []
                for h in range(H):
                    dg = ssp.tile([P, P], f16, name=f"diag{h}",
                                  tag=f"diag{h}")
                    nc.vector.tensor_scalar(
                        out=dg[:], in0=ident16[:],
                        scalar1=expl_self[:, h:h + 1], scalar2=None,
                        op0=mybir.AluOpType.mult)
                    diag_es.append(dg)

                outacc = sop.tile([P, D], f32, name="outacc")

                for r in range(R):
                    if "norel" in ablate or STAGE < 2:
                        continue
                    # ohT[d, b*128+e] = (d == dst_local(slot(b,e)))
                    ps_drl = pdp.tile([P, BT * P], f32, name="ps_drl")
                    base = r * BT * P
                    off = 0
                    while off < BT * P:
                        seg = min(512, BT * P - off)
                        nc.tensor.matmul(
                            out=ps_drl[:, off:off + seg],
                            lhsT=ones1h[:],
                            rhs=drlT_t[:, base + off:base + off + seg],
                            start=True, stop=True)
                        off += seg
                    drlF = ohp.tile([P, BT * P], f16, name="drlF", tag="drlF")
                    nc.scalar.copy(drlF[:], ps_drl[:])
                    ohT = ohp.tile([P, BT * P], f16, name="ohT")
                    nc.vector.tensor_tensor(
                        out=ohT[:], in0=drlF[:], in1=iotaPART[:],
                        op=mybir.AluOpType.is_equal)

                    # ohf[e, b*128+d] = (dloc(b,e) == d), all blocks batched
                    ohf = ohp.tile([P, BT * P], f16, name="ohf")
                    drlB = (drl_t[:, r * BT:(r + 1) * BT]
                            .rearrange("p (b a) -> p b a", a=1)
                            .to_broadcast([P, BT, P]))
                    nc.vector.tensor_tensor(
                        out=ohf[:].rearrange("p (b d) -> p b d", d=P),
                        in0=iotaB, in1=drlB, op=mybir.AluOpType.is_equal)

                    # a_dst per edge via ohT matmuls; asum = a_dst + a_src
                    ps_a = pap.tile([P, BT * 4], f32, name="ps_a")
                    for b in range(BT):
                        nc.tensor.matmul(
                            out=ps_a[:, b * 4:(b + 1) * 4],
                            lhsT=ohT[:, b * P:(b + 1) * P],
                            rhs=twin[:, 266:270],
                            start=True, stop=True)
                    prod = ssp.tile([P, BT * 4], f32, name="prod")
                    nc.vector.tensor_tensor(
                        out=prod[:, 0:B1 * 4], in0=ps_a[:, 0:B1 * 4],
                        in1=G_lo3[:, r * B1:(r + 1) * B1, 262:266],
                        op=mybir.AluOpType.mult)
                    nc.vector.tensor_tensor(
                        out=prod[:, B1 * 4:BT * 4], in0=ps_a[:, B1 * 4:BT * 4],
                        in1=G_hi3[:, r * B2:(r + 1) * B2, 262:266],
                        op=mybir.AluOpType.mult)
                    prod4 = prod[:].rearrange("p (b k) -> p b k", k=4)
                    expl = ssp.tile([P, BT * H], f16, name="expl")
                    nc.vector.tensor_tensor(
                        out=expl[:].rearrange("p (b h) -> p b h", h=H),
                        in0=prod4[:, :, 0:2], in1=prod4[:, :, 2:4],
                        op=mybir.AluOpType.max)

                    if STAGE < 3:
                        continue
                    # ohs_h[e, b, d] = ohf * expl[e, b, h]; agg rhs = raw G
                    expl3 = expl[:].rearrange("p (b h) -> p b h", h=H)
                    ps_rel = pmp.tile([P, 2 * (C + 1)], f32, name="ps_rel")
                    for h in range(H):
                        ohs = gpp.tile([P, BT * P], f16, name="ohs",
                                       tag="ohs")
                        nc.vector.tensor_tensor(
                            out=ohs[:].rearrange("p (b d) -> p b d", d=P),
                            in0=ohf[:].rearrange("p (b d) -> p b d", d=P),
                            in1=expl3[:, :, h:h + 1].to_broadcast([P, BT, P]),
                            op=mybir.AluOpType.mult)
                        for b in range(BT):
                            if b < B1:
                                grhs = G_lo3[:, r * B1 + b,
                                             h * (C + 1):(h + 1) * (C + 1)]
                            else:
                                grhs = G_hi3[:, r * B2 + (b - B1),
                                             h * (C + 1):(h + 1) * (C + 1)]
                            nc.tensor.matmul(
                                out=ps_rel[:, h * (C + 1):(h + 1) * (C + 1)],
                                lhsT=ohs[:, b * P:(b + 1) * P],
                                rhs=grhs,
                                start=(b == 0), stop=False)
                        # self-loop: diag(expl_self_h) @ twin half
                        nc.tensor.matmul(
                            out=ps_rel[:, h * (C + 1):(h + 1) * (C + 1)],
                            lhsT=diag_es[h][:],
                            rhs=twin[:, h * (C + 1):(h + 1) * (C + 1)],
                            start=False, stop=True)

                    # divide + self-loop contribution + accumulate
                    if STAGE < 4:
                        continue
                    s_eps = ssp.tile([P, H], f32, name="s_eps")
                    nc.vector.tensor_scalar(
                        out=s_eps[:].rearrange("p (h a) -> p h a", a=1),
                        in0=ps_rel[:].rearrange("p (h c) -> p h c",
                                                c=C + 1)[:, :, C:C + 1],
                        scalar1=EPS, scalar2=None,
                        op0=mybir.AluOpType.add)
                    recip = ssp.tile([P, H], f32, name="recip")
                    nc.vector.reciprocal(recip[:], s_eps[:])
                    for h in range(H):
                        acc_in = (bias5 if r == 0 else outacc)
                        nc.vector.scalar_tensor_tensor(
                            out=outacc[:, h * C:(h + 1) * C],
                            in0=ps_rel[:, h * (C + 1):h * (C + 1) + C],
                            scalar=recip[:, h:h + 1],
                            in1=acc_in[:, h * C:(h + 1) * C],
                            op0=mybir.AluOpType.mult,
                            op1=mybir.AluOpType.add)
                if "norel" in ablate or STAGE < 4:
                    nc.vector.memset(outacc[:], 0.0)
                nc.sync.dma_start(y[rows, :], outacc[:])
            stack.close()

    nc.finalize()
    return nc


def _wrap16(vals):
    """[n] int array -> 16-partition-wrapped [128, n//16] int16 (replicated)."""
    n = len(vals)
    assert n % 16 == 0
    a = np.asarray(vals, np.int16).reshape(n // 16, 16).T  # [16, n//16]
    return np.tile(a, (8, 1))


def prep_inputs(inputs, ncores):
    x = np.asarray(inputs["x"], dtype=np.float32)
    N = x.shape[0]
    nw_real = -(-N // P)
    NW = -(-nw_real // ncores) * ncores
    w_pc = NW // ncores
    n_tiles = NW
    t_rows = NW * P
    low_cap = min(LOW_CAP, t_rows)
    h0 = t_rows - low_cap

    rels = ["parent", "child", "precede", "follow", "peer"]
    per_rel = []
    for rn in rels:
        ei = np.asarray(inputs[f"edge_index_{rn}"])
        src = ei[0].astype(np.int64)
        dst = ei[1].astype(np.int64)
        order = np.argsort(dst, kind="stable")
        src, dst = src[order], dst[order]
        w_of = dst // P
        cnt = np.bincount(w_of, minlength=NW)
        starts = np.zeros(NW + 1, np.int64)
        np.cumsum(cnt, out=starts[1:])
        per_rel.append((src, dst, starts))

    # global B1/B2 from per-(w,r) counts
    must_lo_max = must_hi_max = tot_max = 0
    for src, dst, starts in per_rel:
        for w in range(NW):
            s, e = starts[w], starts[w + 1]
            sw = src[s:e]
            must_lo_max = max(must_lo_max, int((sw < h0).sum()))
            must_hi_max = max(must_hi_max, int((sw >= low_cap).sum()))
            tot_max = max(tot_max, e - s)
    B1 = max(1, -(-must_lo_max // P))
    B2 = max(1, -(-must_hi_max // P), -(-tot_max // P) - B1)
    while B1 * P < must_lo_max or (tot_max - B1 * P) > B2 * P:
        B1 += 1
    BT = B1 + B2

    lo_idx = np.zeros((NW, R, B1 * P), np.int64)
    hi_idx = np.zeros((NW, R, B2 * P), np.int64)  # pad -> hi row 0 (valid)
    dloc = np.full((NW, R, BT * P), float(P), np.float32)  # sentinel 128
    for r, (src, dst, starts) in enumerate(per_rel):
        for w in range(NW):
            s, e = starts[w], starts[w + 1]
            sw, dw = src[s:e], dst[s:e]
            is_lo = sw < h0
            is_hi = sw >= low_cap
            flex = ~is_lo & ~is_hi
            n_lo_strict = int(is_lo.sum())
            room = B1 * P - n_lo_strict
            fi = np.flatnonzero(flex)
            lo_sel = np.concatenate([np.flatnonzero(is_lo), fi[:room]])
            hi_sel = np.concatenate([np.flatnonzero(is_hi), fi[room:]])
            assert len(lo_sel) <= B1 * P and len(hi_sel) <= B2 * P, (
                w, r, len(lo_sel), len(hi_sel))
            lo_idx[w, r, :len(lo_sel)] = sw[lo_sel]
            hi_idx[w, r, :len(hi_sel)] = sw[hi_sel] - h0
            dloc[w, r, :len(lo_sel)] = (dw[lo_sel] - w * P)
            dloc[w, r, B1 * P:B1 * P + len(hi_sel)] = (dw[hi_sel] - w * P)

    xTf = np.zeros((D, t_rows), np.float16)
    xTf[:, :N] = np.ascontiguousarray(x.T).astype(np.float16)

    iota16 = np.tile(np.arange(P, dtype=np.float16), (P, 1))
    iotac = np.arange(P, dtype=np.float32).reshape(P, 1)

    shared = {
        "xT": xTf,
        "Wsrc": np.ascontiguousarray(np.asarray(inputs["W_src"], np.float32)),
        "Wdst": np.ascontiguousarray(np.asarray(inputs["W_dst"], np.float32)),
        "atts": np.asarray(inputs["att_src"], np.float32).reshape(1, D).copy(),
        "attd": np.asarray(inputs["att_dst"], np.float32).reshape(1, D).copy(),
        "bias_in": np.asarray(inputs["bias"], np.float32).reshape(1, D).copy(),
        "iota16_in": iota16,
        "iotac_in": iotac,
    }

    percore = []
    for c in range(ncores):
        lo16 = np.zeros((w_pc * P, R * B1 * P // 16), np.int16)
        hi16 = np.zeros((w_pc * P, R * B2 * P // 16), np.int16)
        drl16 = np.zeros((w_pc * P, R * BT), np.float16)
        drlT16 = np.zeros((w_pc, R * BT * P), np.float16)
        for wl in range(w_pc):
            w = c * w_pc + wl
            lo_vals = np.concatenate([lo_idx[w, r] for r in range(R)])
            hi_vals = np.concatenate([hi_idx[w, r] for r in range(R)])
            lo16[wl * P:(wl + 1) * P, :] = _wrap16(lo_vals)
            hi16[wl * P:(wl + 1) * P, :] = _wrap16(hi_vals)
            # drl16[p, r*BT+b] = dloc[w, r, b*128+p]
            drl16[wl * P:(wl + 1) * P, :] = dloc[w].reshape(R * BT, P).T
            drlT16[wl, :] = dloc[w].reshape(-1)
        percore.append({
            "lo16": lo16, "hi16": hi16, "drl": drl16, "drlT": drlT16,
            "xT_local": np.ascontiguousarray(
                xTf[:, c * w_pc * P:(c + 1) * w_pc * P]),
        })
    meta = dict(N=N, NW=NW, w_pc=w_pc, n_tiles=n_tiles, B1=B1, B2=B2, h0=h0)
    return meta, shared, percore


def kernel(**inputs):
    global _LAST_RESULT
    from concourse.bass_utils import run_bass_kernel_spmd

    ncores = 8
    meta, shared, percore = prep_inputs(inputs, ncores)
    key = tuple(sorted(meta.items()))
    if key not in _CACHE:
        _CACHE[key] = build_program(
            meta["n_tiles"], meta["w_pc"], meta["B1"], meta["B2"],
            meta["h0"], ncores)
    nc = _CACHE[key]
    in_maps = [dict(shared, **percore[c]) for c in range(ncores)]
    res = run_bass_kernel_spmd(nc, in_maps, core_ids=list(range(ncores)),
                               **_RUN_KWARGS)
    _LAST_RESULT = res
    out = np.concatenate([res.results[c]["y"] for c in range(ncores)], axis=0)
    return np.ascontiguousarray(out[:meta["N"]])

